# revision 5
# baseline (speedup 1.0000x reference)
"""Trainium2 Bass kernel for nn_CLSAwareFFN (router + BlockFFN MLP).

Computes, for hidden_states x [B,S,H], weights W_router [E,H], W_gate [I,H],
W_up [I,H], W_down [H,I]:
    routing_signals = relu(x @ W_router.T)                    [B,S,E]
    output = (silu(x @ W_gate.T) * (x @ W_up.T)) @ W_down.T   [B,S,H]

Strategy: pure data-parallel over the B*S=8192 tokens across 8 NeuronCores
(1024 tokens/core); every core streams the full weights exactly once
(~218 MB/core, well under the matmul time at ~360 GB/s). All layout
transposes/swizzles are done host-side in numpy so every device DMA has
multi-KB contiguous per-partition lines. The kernel works in a transposed
domain (activations stored [feature, token]) so every matmul is a natural
lhsT/rhs pair with the contraction dim on SBUF partitions; matmuls run in
float32r (full PE rate, ~2^-13 relative accuracy).

Fused single pass over i-blocks in groups of 4: gate/up matmuls produce
gateT/upT in PSUM, ScalarE applies SiLU, VectorE multiplies into an SBUF
tile (the group's hp slab, [512, 1024] per group), then the down-projection
matmuls contract the group's 4 i-blocks into 2-bank PSUM tiles which
VectorE flushes into a resident SBUF fp32 accumulator [H, T]. No DRAM
spill of intermediates; PSUM budget: 4 banks gate/up + 4 banks down.
"""

import contextlib
import ctypes
import os
import sys
import types

import numpy as np

import concourse.bass as bass
import concourse.mybir as mybir
import concourse.tile as tile
from concourse import bacc
from concourse.bass import ds
from concourse.bass_utils import run_bass_kernel_spmd

# Problem shape (hardcoded per contest contract).
B, S, H, I, E = 4, 2048, 2048, 8192, 64
N_CORES = 8
T_TOT = B * S            # 8192 tokens
T = T_TOT // N_CORES     # 1024 tokens per core

P = 128
HO = H // P              # 16 h-tiles
IO = I // P              # 64 i-tiles
HC = H // P              # 16 output (down) chunks of 128
TH = 512                 # moving free dim per matmul
NTH = T // TH            # 2 token-halves per core
GRP = 4                  # i-blocks fused per down-accumulation group
NG = IO // GRP           # 16 groups
QH = 512                 # hh span per wd tile
NQ = H // QH             # 4 quads
QC = QH // P             # 4 chunks per quad

f32 = mybir.dt.float32
f32r = mybir.dt.float32r
AF = mybir.ActivationFunctionType

_CACHE = {}


def _ensure_axon_ntff_hook():
    """Provide antenv.axon_hooks when the trimmed client image lacks it, so
    run_bass_kernel_spmd(trace=True) (or BASS_TRACE=1) degrades gracefully
    instead of raising ModuleNotFoundError."""
    try:
        import antenv.axon_hooks  # noqa: F401
        return
    except ImportError:
        pass

    hook = None
    so_path = "/opt/axon/libaxon_pjrt.so"
    if os.path.exists(so_path):
        try:
            lib = ctypes.CDLL(so_path)
            if hasattr(lib, "axon_start_nrt_profile"):
                lib.axon_start_nrt_profile.argtypes = [
                    ctypes.POINTER(ctypes.c_int64), ctypes.c_size_t]
                lib.axon_start_nrt_profile.restype = ctypes.c_int64
                lib.axon_stop_nrt_profile.argtypes = [ctypes.c_char_p]
                lib.axon_stop_nrt_profile.restype = ctypes.c_int64

                @contextlib.contextmanager
                def _hook(output_dir, device_ids):
                    import jax
                    jax.devices()
                    if device_ids:
                        ids = (ctypes.c_int64 * len(device_ids))(*device_ids)
                        rc = lib.axon_start_nrt_profile(ids, len(device_ids))
                    else:
                        rc = lib.axon_start_nrt_profile(None, 0)
                    if rc != 0:
                        raise RuntimeError(f"axon_start_nrt_profile rc={rc}")
                    try:
                        yield
                    finally:
                        n = lib.axon_stop_nrt_profile(str(output_dir).encode())
                        print(f"ntff profile: {n} file(s) -> {output_dir}",
                              file=sys.stderr)

                hook = _hook
        except OSError:
            pass

    import antenv
    mod = types.ModuleType("antenv.axon_hooks")
    mod.get_axon_ntff_profile_hook = lambda: hook
    mod.set_axon_ntff_profile_hook = lambda h: None
    antenv.axon_hooks = mod
    sys.modules["antenv.axon_hooks"] = mod


_ensure_axon_ntff_hook()


def _build():
    nc = bacc.Bacc("TRN2", target_bir_lowering=False, debug=False,
                   num_devices=N_CORES)

    # Host-swizzled layouts (see kernel()): per-partition lines are
    # contiguous multi-KB chunks.
    xS = nc.dram_tensor("xS", [P, HO, T], f32, kind="ExternalInput")
    WgS = nc.dram_tensor("WgS", [IO, P, HO, P], f32, kind="ExternalInput")
    WuS = nc.dram_tensor("WuS", [IO, P, HO, P], f32, kind="ExternalInput")
    WdS = nc.dram_tensor("WdS", [IO, P, H], f32, kind="ExternalInput")
    WrS = nc.dram_tensor("WrS", [P, HO, E], f32, kind="ExternalInput")
    outT = nc.dram_tensor("outT", [H, T], f32, kind="ExternalOutput")
    rT = nc.dram_tensor("rT", [E, T], f32, kind="ExternalOutput")

    xS_r = xS.ap().bitcast(f32r)
    WgS_r = WgS.ap().bitcast(f32r)
    WuS_r = WuS.ap().bitcast(f32r)
    WdS_r = WdS.ap().bitcast(f32r)
    WrS_r = WrS.ap().bitcast(f32r)

    with tile.TileContext(nc) as tc:
        with tc.tile_pool(name="res", bufs=1) as res, \
             tc.tile_pool(name="wts", bufs=2) as wts, \
             tc.tile_pool(name="work", bufs=2) as work, \
             tc.tile_pool(name="pp", bufs=1, space="PSUM") as pp:

            x_sb = res.tile([P, HO, T], f32r)
            nc.sync.dma_start(x_sb, xS_r)

            acc = [res.tile([P, T], f32, tag=f"acc{c}", name=f"acc{c}")
                   for c in range(HC)]

            # ---- router ----
            wr_sb = res.tile([P, HO, E], f32r)
            nc.sync.dma_start(wr_sb, WrS_r)
            for th in range(NTH):
                r_ps = pp.tile([E, TH], f32, tag=f"gps{th}", name="r_ps")
                for k in range(HO):
                    nc.tensor.matmul(r_ps, wr_sb[:, k],
                                     x_sb[:, k, ds(th * TH, TH)],
                                     start=(k == 0), stop=(k == HO - 1))
                r_sb = work.tile([E, TH], f32, tag="rsb")
                nc.scalar.activation(r_sb, r_ps, AF.Relu)
                nc.sync.dma_start(rT.ap()[:, ds(th * TH, TH)], r_sb)

            # ---- fused gate/up + down ----
            for g in range(NG):
                hp_sb = []
                for j in range(GRP):
                    ib = g * GRP + j
                    wg_sb = wts.tile([P, HO, P], f32r, tag="wg")
                    wu_sb = wts.tile([P, HO, P], f32r, tag="wu")
                    nc.sync.dma_start(wg_sb, WgS_r[ib])
                    nc.sync.dma_start(wu_sb, WuS_r[ib])
                    g_ps = [pp.tile([P, TH], f32, tag=f"gps{th}",
                                    name=f"g_ps{th}") for th in range(NTH)]
                    u_ps = [pp.tile([P, TH], f32, tag=f"ups{th}",
                                    name=f"u_ps{th}") for th in range(NTH)]
                    for k in range(HO):
                        for th in range(NTH):
                            nc.tensor.matmul(g_ps[th], wg_sb[:, k],
                                             x_sb[:, k, ds(th * TH, TH)],
                                             start=(k == 0), stop=(k == HO - 1))
                    for k in range(HO):
                        for th in range(NTH):
                            nc.tensor.matmul(u_ps[th], wu_sb[:, k],
                                             x_sb[:, k, ds(th * TH, TH)],
                                             start=(k == 0), stop=(k == HO - 1))
                    hp = work.tile([P, T], f32r, tag="hp", bufs=5, name="hp")
                    for th in range(NTH):
                        sg_sb = work.tile([P, TH], f32, tag="sg")
                        nc.scalar.activation(sg_sb, g_ps[th], AF.Silu)
                        nc.vector.tensor_mul(hp[:, ds(th * TH, TH)],
                                             sg_sb, u_ps[th])
                    hp_sb.append(hp)

                for q in range(NQ):
                    wd_sb = []
                    for j in range(GRP):
                        ib = g * GRP + j
                        wd = wts.tile([P, QH], f32r, tag="wd", bufs=6,
                                      name="wd")
                        nc.sync.dma_start(wd, WdS_r[ib][:, ds(q * QH, QH)])
                        wd_sb.append(wd)
                    for c in range(QC):
                        d_ps = pp.tile([P, T], f32, tag=f"dps{c % 2}",
                                       name="d_ps")
                        for j in range(GRP):
                            for th in range(NTH):
                                nc.tensor.matmul(
                                    d_ps[:, ds(th * TH, TH)],
                                    wd_sb[j][:, ds(c * P, P)],
                                    hp_sb[j][:, ds(th * TH, TH)],
                                    start=(j == 0), stop=(j == GRP - 1))
                        cg = q * QC + c
                        if g == 0:
                            nc.vector.tensor_copy(acc[cg], d_ps)
                        else:
                            nc.vector.tensor_add(acc[cg], acc[cg], d_ps)
                        if g == NG - 1:
                            nc.sync.dma_start(outT.ap()[ds(cg * P, P), :],
                                              acc[cg])

    nc.compile()
    return nc


def _get_nc():
    if "nc" not in _CACHE:
        _CACHE["nc"] = _build()
    return _CACHE["nc"]


def kernel(hidden_states, W_router, W_gate, W_up, W_down, _trace=False):
    nc = _get_nc()

    x = np.asarray(hidden_states, dtype=np.float32).reshape(T_TOT, H)
    # xS[core][p, ko, t] = x[core*T + t, ko*P + p]
    xS = np.ascontiguousarray(
        x.reshape(N_CORES, T, HO, P).transpose(0, 3, 2, 1))
    # WgS[ib, p, ko, ii] = W_gate[ib*P + ii, ko*P + p]
    WgS = np.ascontiguousarray(
        np.asarray(W_gate, dtype=np.float32).reshape(IO, P, HO, P)
        .transpose(0, 3, 2, 1))
    WuS = np.ascontiguousarray(
        np.asarray(W_up, dtype=np.float32).reshape(IO, P, HO, P)
        .transpose(0, 3, 2, 1))
    # WdS[ib, p, h] = W_down[h, ib*P + p]
    WdS = np.ascontiguousarray(
        np.asarray(W_down, dtype=np.float32).T).reshape(IO, P, H)
    # WrS[p, ko, e] = W_router[e, ko*P + p]
    WrS = np.ascontiguousarray(
        np.asarray(W_router, dtype=np.float32).reshape(E, HO, P)
        .transpose(2, 1, 0))

    in_maps = []
    for c in range(N_CORES):
        in_maps.append({
            "xS": xS[c], "WgS": WgS, "WuS": WuS, "WdS": WdS, "WrS": WrS,
        })

    res = run_bass_kernel_spmd(nc, in_maps, core_ids=list(range(N_CORES)),
                               trace=_trace)
    _CACHE["last_result"] = res

    outT_full = np.empty((H, T_TOT), dtype=np.float32)
    rT_full = np.empty((E, T_TOT), dtype=np.float32)
    for c in range(N_CORES):
        outT_full[:, c * T:(c + 1) * T] = res.results[c]["outT"]
        rT_full[:, c * T:(c + 1) * T] = res.results[c]["rT"]

    output = np.ascontiguousarray(outT_full.T).reshape(B, S, H)
    routing_signals = np.ascontiguousarray(rT_full.T).reshape(B, S, E)
    return output, routing_signals


# revision 6
# speedup vs baseline: 1.0804x; 1.0804x over previous
"""Trainium2 Bass kernel for nn_CLSAwareFFN (router + BlockFFN MLP).

Computes, for hidden_states x [B,S,H], weights W_router [E,H], W_gate [I,H],
W_up [I,H], W_down [H,I]:
    routing_signals = relu(x @ W_router.T)                    [B,S,E]
    output = (silu(x @ W_gate.T) * (x @ W_up.T)) @ W_down.T   [B,S,H]

Strategy: pure data-parallel over the B*S=8192 tokens across 8 NeuronCores
(1024 tokens/core); every core streams the full weights exactly once
(~218 MB/core, well under the matmul time at ~360 GB/s). All layout
transposes/swizzles are done host-side in numpy so every device DMA has
multi-KB contiguous per-partition lines. The kernel works in a transposed
domain (activations stored [feature, token]) so every matmul is a natural
lhsT/rhs pair with the contraction dim on SBUF partitions; matmuls run in
float32r (full PE rate, ~2^-13 relative accuracy).

Fused single pass over i-blocks in groups of 4: gate/up matmuls produce
gateT/upT in PSUM, ScalarE applies SiLU, VectorE multiplies into an SBUF
tile (the group's hp slab, [512, 1024] per group), then the down-projection
matmuls contract the group's 4 i-blocks into 2-bank PSUM tiles which
VectorE flushes into a resident SBUF fp32 accumulator [H, T]. No DRAM
spill of intermediates; PSUM budget: 4 banks gate/up + 4 banks down.
"""

import contextlib
import ctypes
import os
import sys
import types

import numpy as np

import concourse.bass as bass
import concourse.mybir as mybir
import concourse.tile as tile
from concourse import bacc
from concourse.bass import ds
from concourse.bass_utils import run_bass_kernel_spmd

# Problem shape (hardcoded per contest contract).
B, S, H, I, E = 4, 2048, 2048, 8192, 64
N_CORES = 8
T_TOT = B * S            # 8192 tokens
T = T_TOT // N_CORES     # 1024 tokens per core

P = 128
HO = H // P              # 16 h-tiles
IO = I // P              # 64 i-tiles
HC = H // P              # 16 output (down) chunks of 128
TH = 512                 # moving free dim per matmul
NTH = T // TH            # 2 token-halves per core
GRP = 4                  # i-blocks fused per down-accumulation group
NG = IO // GRP           # 16 groups
QH = 512                 # hh span per wd tile
NQ = H // QH             # 4 quads
QC = QH // P             # 4 chunks per quad

f32 = mybir.dt.float32
f32r = mybir.dt.float32r
AF = mybir.ActivationFunctionType

_CACHE = {}


def _ensure_axon_ntff_hook():
    """Provide antenv.axon_hooks when the trimmed client image lacks it, so
    run_bass_kernel_spmd(trace=True) (or BASS_TRACE=1) degrades gracefully
    instead of raising ModuleNotFoundError."""
    try:
        import antenv.axon_hooks  # noqa: F401
        return
    except ImportError:
        pass

    hook = None
    so_path = "/opt/axon/libaxon_pjrt.so"
    if os.path.exists(so_path):
        try:
            lib = ctypes.CDLL(so_path)
            if hasattr(lib, "axon_start_nrt_profile"):
                lib.axon_start_nrt_profile.argtypes = [
                    ctypes.POINTER(ctypes.c_int64), ctypes.c_size_t]
                lib.axon_start_nrt_profile.restype = ctypes.c_int64
                lib.axon_stop_nrt_profile.argtypes = [ctypes.c_char_p]
                lib.axon_stop_nrt_profile.restype = ctypes.c_int64

                @contextlib.contextmanager
                def _hook(output_dir, device_ids):
                    import jax
                    jax.devices()
                    if device_ids:
                        ids = (ctypes.c_int64 * len(device_ids))(*device_ids)
                        rc = lib.axon_start_nrt_profile(ids, len(device_ids))
                    else:
                        rc = lib.axon_start_nrt_profile(None, 0)
                    if rc != 0:
                        raise RuntimeError(f"axon_start_nrt_profile rc={rc}")
                    try:
                        yield
                    finally:
                        n = lib.axon_stop_nrt_profile(str(output_dir).encode())
                        print(f"ntff profile: {n} file(s) -> {output_dir}",
                              file=sys.stderr)

                hook = _hook
        except OSError:
            pass

    import antenv
    mod = types.ModuleType("antenv.axon_hooks")
    mod.get_axon_ntff_profile_hook = lambda: hook
    mod.set_axon_ntff_profile_hook = lambda h: None
    antenv.axon_hooks = mod
    sys.modules["antenv.axon_hooks"] = mod


_ensure_axon_ntff_hook()


def _build():
    nc = bacc.Bacc("TRN2", target_bir_lowering=False, debug=False,
                   num_devices=N_CORES)

    # Host-swizzled layouts (see kernel()): per-partition lines are
    # contiguous multi-KB chunks.
    xS = nc.dram_tensor("xS", [NTH, P, HO, TH], f32, kind="ExternalInput")
    WgS = nc.dram_tensor("WgS", [IO, P, HO, P], f32, kind="ExternalInput")
    WuS = nc.dram_tensor("WuS", [IO, P, HO, P], f32, kind="ExternalInput")
    WdS = nc.dram_tensor("WdS", [IO, P, H], f32, kind="ExternalInput")
    WrS = nc.dram_tensor("WrS", [P, HO, E], f32, kind="ExternalInput")
    outT = nc.dram_tensor("outT", [H, T], f32, kind="ExternalOutput")
    rT = nc.dram_tensor("rT", [E, T], f32, kind="ExternalOutput")

    xS_r = xS.ap().bitcast(f32r)
    WgS_r = WgS.ap().bitcast(f32r)
    WuS_r = WuS.ap().bitcast(f32r)
    WdS_r = WdS.ap().bitcast(f32r)
    WrS_r = WrS.ap().bitcast(f32r)

    with tile.TileContext(nc) as tc:
        with tc.tile_pool(name="res", bufs=1) as res, \
             tc.tile_pool(name="wts", bufs=2) as wts, \
             tc.tile_pool(name="work", bufs=2) as work, \
             tc.tile_pool(name="pp", bufs=1, space="PSUM") as pp:

            x_sb = []
            for th in range(NTH):
                x_h = res.tile([P, HO, TH], f32r, tag=f"x{th}", name=f"x{th}")
                nc.sync.dma_start(x_h, xS_r[th])
                x_sb.append(x_h)

            acc = [res.tile([P, T], f32, tag=f"acc{c}", name=f"acc{c}")
                   for c in range(HC)]

            # ---- router ----
            wr_sb = res.tile([P, HO, E], f32r)
            nc.sync.dma_start(wr_sb, WrS_r)
            for th in range(NTH):
                r_ps = pp.tile([E, TH], f32, tag=f"gps{th}", name="r_ps")
                for k in range(HO):
                    nc.tensor.matmul(r_ps, wr_sb[:, k],
                                     x_sb[th][:, k],
                                     start=(k == 0), stop=(k == HO - 1))
                r_sb = work.tile([E, TH], f32, tag="rsb", bufs=1)
                nc.scalar.activation(r_sb, r_ps, AF.Relu)
                nc.sync.dma_start(rT.ap()[:, ds(th * TH, TH)], r_sb)

            # ---- fused gate/up + down ----
            for g in range(NG):
                hp_sb = []
                for j in range(GRP):
                    ib = g * GRP + j
                    wg_sb = wts.tile([P, HO, P], f32r, tag="wg")
                    wu_sb = wts.tile([P, HO, P], f32r, tag="wu")
                    nc.sync.dma_start(wg_sb, WgS_r[ib])
                    nc.sync.dma_start(wu_sb, WuS_r[ib])
                    g_ps = [pp.tile([P, TH], f32, tag=f"gps{th}",
                                    name=f"g_ps{th}") for th in range(NTH)]
                    u_ps = [pp.tile([P, TH], f32, tag=f"ups{th}",
                                    name=f"u_ps{th}") for th in range(NTH)]
                    for k in range(HO):
                        for th in range(NTH):
                            nc.tensor.matmul(g_ps[th], wg_sb[:, k],
                                             x_sb[th][:, k],
                                             start=(k == 0), stop=(k == HO - 1))
                    for k in range(HO):
                        for th in range(NTH):
                            nc.tensor.matmul(u_ps[th], wu_sb[:, k],
                                             x_sb[th][:, k],
                                             start=(k == 0), stop=(k == HO - 1))
                    hp = work.tile([P, T], f32r, tag="hp", bufs=5, name="hp")
                    for th in range(NTH):
                        sg_sb = work.tile([P, TH], f32, tag="sg", bufs=1)
                        nc.scalar.activation(sg_sb, g_ps[th], AF.Silu)
                        nc.vector.tensor_mul(hp[:, ds(th * TH, TH)],
                                             sg_sb, u_ps[th])
                    hp_sb.append(hp)

                for q in range(NQ):
                    wd_sb = []
                    for j in range(GRP):
                        ib = g * GRP + j
                        wd = wts.tile([P, QH], f32r, tag="wd", bufs=8,
                                      name="wd")
                        nc.sync.dma_start(wd, WdS_r[ib][:, ds(q * QH, QH)])
                        wd_sb.append(wd)
                    for c in range(QC):
                        d_ps = pp.tile([P, T], f32, tag=f"dps{c % 2}",
                                       name="d_ps")
                        for j in range(GRP):
                            for th in range(NTH):
                                nc.tensor.matmul(
                                    d_ps[:, ds(th * TH, TH)],
                                    wd_sb[j][:, ds(c * P, P)],
                                    hp_sb[j][:, ds(th * TH, TH)],
                                    start=(j == 0), stop=(j == GRP - 1))
                        cg = q * QC + c
                        if g == 0:
                            nc.vector.tensor_copy(acc[cg], d_ps)
                        else:
                            nc.vector.tensor_add(acc[cg], acc[cg], d_ps)
                        if g == NG - 1:
                            nc.sync.dma_start(outT.ap()[ds(cg * P, P), :],
                                              acc[cg])

    nc.compile()
    return nc


def _get_nc():
    if "nc" not in _CACHE:
        _CACHE["nc"] = _build()
    return _CACHE["nc"]


def kernel(hidden_states, W_router, W_gate, W_up, W_down, _trace=False):
    nc = _get_nc()

    x = np.asarray(hidden_states, dtype=np.float32).reshape(T_TOT, H)
    # xS[core][p, ko, t] = x[core*T + t, ko*P + p]
    # xS[core][th, p, ko, t] = x[core*T + th*TH + t, ko*P + p]
    xS = np.ascontiguousarray(
        x.reshape(N_CORES, NTH, TH, HO, P).transpose(0, 1, 4, 3, 2))
    # WgS[ib, p, ko, ii] = W_gate[ib*P + ii, ko*P + p]
    WgS = np.ascontiguousarray(
        np.asarray(W_gate, dtype=np.float32).reshape(IO, P, HO, P)
        .transpose(0, 3, 2, 1))
    WuS = np.ascontiguousarray(
        np.asarray(W_up, dtype=np.float32).reshape(IO, P, HO, P)
        .transpose(0, 3, 2, 1))
    # WdS[ib, p, h] = W_down[h, ib*P + p]
    WdS = np.ascontiguousarray(
        np.asarray(W_down, dtype=np.float32).T).reshape(IO, P, H)
    # WrS[p, ko, e] = W_router[e, ko*P + p]
    WrS = np.ascontiguousarray(
        np.asarray(W_router, dtype=np.float32).reshape(E, HO, P)
        .transpose(2, 1, 0))

    in_maps = []
    for c in range(N_CORES):
        in_maps.append({
            "xS": xS[c], "WgS": WgS, "WuS": WuS, "WdS": WdS, "WrS": WrS,
        })

    res = run_bass_kernel_spmd(nc, in_maps, core_ids=list(range(N_CORES)),
                               trace=_trace)
    _CACHE["last_result"] = res

    outT_full = np.empty((H, T_TOT), dtype=np.float32)
    rT_full = np.empty((E, T_TOT), dtype=np.float32)
    for c in range(N_CORES):
        outT_full[:, c * T:(c + 1) * T] = res.results[c]["outT"]
        rT_full[:, c * T:(c + 1) * T] = res.results[c]["rT"]

    output = np.ascontiguousarray(outT_full.T).reshape(B, S, H)
    routing_signals = np.ascontiguousarray(rT_full.T).reshape(B, S, E)
    return output, routing_signals


# revision 7
# speedup vs baseline: 1.0826x; 1.0020x over previous
"""Trainium2 Bass kernel for nn_CLSAwareFFN (router + BlockFFN MLP).

Computes, for hidden_states x [B,S,H], weights W_router [E,H], W_gate [I,H],
W_up [I,H], W_down [H,I]:
    routing_signals = relu(x @ W_router.T)                    [B,S,E]
    output = (silu(x @ W_gate.T) * (x @ W_up.T)) @ W_down.T   [B,S,H]

Strategy: pure data-parallel over the B*S=8192 tokens across 8 NeuronCores
(1024 tokens/core); every core streams the full weights exactly once
(~218 MB/core, well under the matmul time at ~360 GB/s). All layout
transposes/swizzles are done host-side in numpy so every device DMA has
multi-KB contiguous per-partition lines. The kernel works in a transposed
domain (activations stored [feature, token]) so every matmul is a natural
lhsT/rhs pair with the contraction dim on SBUF partitions; matmuls run in
float32r (full PE rate, ~2^-13 relative accuracy).

Fused single pass over i-blocks in groups of 4: gate/up matmuls produce
gateT/upT in PSUM, ScalarE applies SiLU, VectorE multiplies into an SBUF
tile (the group's hp slab, [512, 1024] per group), then the down-projection
matmuls contract the group's 4 i-blocks into 2-bank PSUM tiles which
VectorE flushes into a resident SBUF fp32 accumulator [H, T]. No DRAM
spill of intermediates; PSUM budget: 4 banks gate/up + 4 banks down.
"""

import contextlib
import ctypes
import os
import sys
import types

import numpy as np

import concourse.bass as bass
import concourse.mybir as mybir
import concourse.tile as tile
from concourse import bacc
from concourse.bass import ds
from concourse.bass_utils import run_bass_kernel_spmd

# Problem shape (hardcoded per contest contract).
B, S, H, I, E = 4, 2048, 2048, 8192, 64
N_CORES = 8
T_TOT = B * S            # 8192 tokens
T = T_TOT // N_CORES     # 1024 tokens per core

P = 128
HO = H // P              # 16 h-tiles
IO = I // P              # 64 i-tiles
HC = H // P              # 16 output (down) chunks of 128
TH = 512                 # moving free dim per matmul
NTH = T // TH            # 2 token-halves per core
GRP = 4                  # i-blocks fused per down-accumulation group
NG = IO // GRP           # 16 groups
QH = 512                 # hh span per wd tile
NQ = H // QH             # 4 quads
QC = QH // P             # 4 chunks per quad

f32 = mybir.dt.float32
f32r = mybir.dt.float32r
AF = mybir.ActivationFunctionType

_CACHE = {}


def _ensure_axon_ntff_hook():
    """Provide antenv.axon_hooks when the trimmed client image lacks it, so
    run_bass_kernel_spmd(trace=True) (or BASS_TRACE=1) degrades gracefully
    instead of raising ModuleNotFoundError."""
    try:
        import antenv.axon_hooks  # noqa: F401
        return
    except ImportError:
        pass

    hook = None
    so_path = "/opt/axon/libaxon_pjrt.so"
    if os.path.exists(so_path):
        try:
            lib = ctypes.CDLL(so_path)
            if hasattr(lib, "axon_start_nrt_profile"):
                lib.axon_start_nrt_profile.argtypes = [
                    ctypes.POINTER(ctypes.c_int64), ctypes.c_size_t]
                lib.axon_start_nrt_profile.restype = ctypes.c_int64
                lib.axon_stop_nrt_profile.argtypes = [ctypes.c_char_p]
                lib.axon_stop_nrt_profile.restype = ctypes.c_int64

                @contextlib.contextmanager
                def _hook(output_dir, device_ids):
                    import jax
                    jax.devices()
                    if device_ids:
                        ids = (ctypes.c_int64 * len(device_ids))(*device_ids)
                        rc = lib.axon_start_nrt_profile(ids, len(device_ids))
                    else:
                        rc = lib.axon_start_nrt_profile(None, 0)
                    if rc != 0:
                        raise RuntimeError(f"axon_start_nrt_profile rc={rc}")
                    try:
                        yield
                    finally:
                        n = lib.axon_stop_nrt_profile(str(output_dir).encode())
                        print(f"ntff profile: {n} file(s) -> {output_dir}",
                              file=sys.stderr)

                hook = _hook
        except OSError:
            pass

    import antenv
    mod = types.ModuleType("antenv.axon_hooks")
    mod.get_axon_ntff_profile_hook = lambda: hook
    mod.set_axon_ntff_profile_hook = lambda h: None
    antenv.axon_hooks = mod
    sys.modules["antenv.axon_hooks"] = mod


_ensure_axon_ntff_hook()


def _build():
    nc = bacc.Bacc("TRN2", target_bir_lowering=False, debug=False,
                   num_devices=N_CORES)

    # Host-swizzled layouts (see kernel()): per-partition lines are
    # contiguous multi-KB chunks.
    xS = nc.dram_tensor("xS", [NTH, P, HO, TH], f32, kind="ExternalInput")
    WgS = nc.dram_tensor("WgS", [IO, P, HO, P], f32, kind="ExternalInput")
    WuS = nc.dram_tensor("WuS", [IO, P, HO, P], f32, kind="ExternalInput")
    WdS = nc.dram_tensor("WdS", [IO, P, H], f32, kind="ExternalInput")
    WrS = nc.dram_tensor("WrS", [P, HO, E], f32, kind="ExternalInput")
    outT = nc.dram_tensor("outT", [H, T], f32, kind="ExternalOutput")
    rT = nc.dram_tensor("rT", [E, T], f32, kind="ExternalOutput")

    xS_r = xS.ap().bitcast(f32r)
    WgS_r = WgS.ap().bitcast(f32r)
    WuS_r = WuS.ap().bitcast(f32r)
    WdS_r = WdS.ap().bitcast(f32r)
    WrS_r = WrS.ap().bitcast(f32r)

    with tile.TileContext(nc) as tc:
        with tc.tile_pool(name="res", bufs=1) as res, \
             tc.tile_pool(name="wts", bufs=2) as wts, \
             tc.tile_pool(name="work", bufs=2) as work, \
             tc.tile_pool(name="pp", bufs=1, space="PSUM") as pp:

            wr_sb = res.tile([P, HO, E], f32r)
            nc.sync.dma_start(wr_sb, WrS_r)

            # Per-k-tile x loads so the first matmuls only wait on 256 KB.
            x_sb = []
            for th in range(NTH):
                x_h = res.tile([P, HO, TH], f32r, tag=f"x{th}", name=f"x{th}")
                for k in range(HO):
                    nc.sync.dma_start(x_h[:, k, :], xS_r[th, :, k, :])
                x_sb.append(x_h)

            acc = [res.tile([P, T], f32, tag=f"acc{c}", name=f"acc{c}")
                   for c in range(HC)]

            # ---- router ----
            for th in range(NTH):
                r_ps = pp.tile([E, TH], f32, tag=f"gps{th}", name="r_ps")
                for k in range(HO):
                    nc.tensor.matmul(r_ps, wr_sb[:, k],
                                     x_sb[th][:, k],
                                     start=(k == 0), stop=(k == HO - 1))
                r_sb = work.tile([E, TH], f32, tag="rsb", bufs=1)
                nc.scalar.activation(r_sb, r_ps, AF.Relu)
                nc.sync.dma_start(rT.ap()[:, ds(th * TH, TH)], r_sb)

            # ---- fused gate/up + down ----
            for g in range(NG):
                hp_sb = []
                for j in range(GRP):
                    ib = g * GRP + j
                    wg_sb = wts.tile([P, HO, P], f32r, tag="wg")
                    wu_sb = wts.tile([P, HO, P], f32r, tag="wu")
                    nc.sync.dma_start(wg_sb, WgS_r[ib])
                    nc.sync.dma_start(wu_sb, WuS_r[ib])
                    g_ps = [pp.tile([P, TH], f32, tag=f"gps{th}",
                                    name=f"g_ps{th}") for th in range(NTH)]
                    u_ps = [pp.tile([P, TH], f32, tag=f"ups{th}",
                                    name=f"u_ps{th}") for th in range(NTH)]
                    for k in range(HO):
                        for th in range(NTH):
                            nc.tensor.matmul(g_ps[th], wg_sb[:, k],
                                             x_sb[th][:, k],
                                             start=(k == 0), stop=(k == HO - 1))
                    for k in range(HO):
                        for th in range(NTH):
                            nc.tensor.matmul(u_ps[th], wu_sb[:, k],
                                             x_sb[th][:, k],
                                             start=(k == 0), stop=(k == HO - 1))
                    hp = work.tile([P, T], f32r, tag="hp", bufs=5, name="hp")
                    for th in range(NTH):
                        sg_sb = work.tile([P, TH], f32, tag="sg", bufs=1)
                        nc.scalar.activation(sg_sb, g_ps[th], AF.Silu)
                        nc.vector.tensor_mul(hp[:, ds(th * TH, TH)],
                                             sg_sb, u_ps[th])
                    hp_sb.append(hp)

                for q in range(NQ):
                    wd_sb = []
                    for j in range(GRP):
                        ib = g * GRP + j
                        wd = wts.tile([P, QH], f32r, tag="wd", bufs=8,
                                      name="wd")
                        nc.sync.dma_start(wd, WdS_r[ib][:, ds(q * QH, QH)])
                        wd_sb.append(wd)
                    for c in range(QC):
                        d_ps = pp.tile([P, T], f32, tag=f"dps{c % 2}",
                                       name="d_ps")
                        for j in range(GRP):
                            for th in range(NTH):
                                nc.tensor.matmul(
                                    d_ps[:, ds(th * TH, TH)],
                                    wd_sb[j][:, ds(c * P, P)],
                                    hp_sb[j][:, ds(th * TH, TH)],
                                    start=(j == 0), stop=(j == GRP - 1))
                        cg = q * QC + c
                        if g == 0:
                            nc.vector.tensor_copy(acc[cg], d_ps)
                        else:
                            nc.vector.tensor_add(acc[cg], acc[cg], d_ps)
                        if g == NG - 1:
                            nc.sync.dma_start(outT.ap()[ds(cg * P, P), :],
                                              acc[cg])

    nc.compile()
    return nc


def _get_nc():
    if "nc" not in _CACHE:
        _CACHE["nc"] = _build()
    return _CACHE["nc"]


def kernel(hidden_states, W_router, W_gate, W_up, W_down, _trace=False):
    nc = _get_nc()

    x = np.asarray(hidden_states, dtype=np.float32).reshape(T_TOT, H)
    # xS[core][p, ko, t] = x[core*T + t, ko*P + p]
    # xS[core][th, p, ko, t] = x[core*T + th*TH + t, ko*P + p]
    xS = np.ascontiguousarray(
        x.reshape(N_CORES, NTH, TH, HO, P).transpose(0, 1, 4, 3, 2))
    # WgS[ib, p, ko, ii] = W_gate[ib*P + ii, ko*P + p]
    WgS = np.ascontiguousarray(
        np.asarray(W_gate, dtype=np.float32).reshape(IO, P, HO, P)
        .transpose(0, 3, 2, 1))
    WuS = np.ascontiguousarray(
        np.asarray(W_up, dtype=np.float32).reshape(IO, P, HO, P)
        .transpose(0, 3, 2, 1))
    # WdS[ib, p, h] = W_down[h, ib*P + p]
    WdS = np.ascontiguousarray(
        np.asarray(W_down, dtype=np.float32).T).reshape(IO, P, H)
    # WrS[p, ko, e] = W_router[e, ko*P + p]
    WrS = np.ascontiguousarray(
        np.asarray(W_router, dtype=np.float32).reshape(E, HO, P)
        .transpose(2, 1, 0))

    in_maps = []
    for c in range(N_CORES):
        in_maps.append({
            "xS": xS[c], "WgS": WgS, "WuS": WuS, "WdS": WdS, "WrS": WrS,
        })

    res = run_bass_kernel_spmd(nc, in_maps, core_ids=list(range(N_CORES)),
                               trace=_trace)
    _CACHE["last_result"] = res

    outT_full = np.empty((H, T_TOT), dtype=np.float32)
    rT_full = np.empty((E, T_TOT), dtype=np.float32)
    for c in range(N_CORES):
        outT_full[:, c * T:(c + 1) * T] = res.results[c]["outT"]
        rT_full[:, c * T:(c + 1) * T] = res.results[c]["rT"]

    output = np.ascontiguousarray(outT_full.T).reshape(B, S, H)
    routing_signals = np.ascontiguousarray(rT_full.T).reshape(B, S, E)
    return output, routing_signals
